# revision 23
# baseline (speedup 1.0000x reference)
"""Distributed Trainium2 Bass kernel for multi-head attention w/ partial RoPE.

Reference math (B=4, N=2048, DIM=1024, H=16, dh=64, rot=32):
  q,k,v = x@Wq, x@Wk, x@Wv (per-head views), partial rope on first 32 ch of
  q,k,v; attn = softmax(q k^T * dh^-0.5); out = (attn @ v) @ Wo + bo + x.
  Returns (out, attn).

Sharding: tensor-parallel over heads across 8 NeuronCores, 2 heads/core.
Per core: project full x against its 128 columns of Wq/Wk/Wv, run attention
for its 2 heads (attn slice written bf16), AllGather per-head attention
outputs (bf16, per batch), then compute a 128-column slice of the output
projection (+bias+residual).
"""
import os
import sys

sys.path.insert(0, "/opt/trn_rl_repo")


import numpy as np
import ml_dtypes

import concourse.bass as bass
import concourse.tile as tile
from concourse import bacc, mybir
from concourse.bass_utils import run_bass_kernel_spmd

BF = mybir.dt.bfloat16
F32 = mybir.dt.float32
AF = mybir.ActivationFunctionType

N_CORES = 8
B, N, DIM, HEADS, ROT = 4, 2048, 1024, 16, 32
DH = DIM // HEADS            # 64
TOK = B * N                  # 8192
SCALE = DH ** -0.5           # 0.125
NKT = DIM // 128             # 8 contraction tiles for projections
NCH = TOK // 512             # 16 token chunks for projections
NIT = N // 128               # 16 i-tiles per round
NR = 2 * B                   # 8 attention rounds (batch x local head)

_BUILT = None


def build():
    nc = bacc.Bacc("TRN2", target_bir_lowering=False, debug=False, num_devices=N_CORES)

    # ---- parameters (per-core shards prepared on host) ----
    xT = nc.declare_dram_parameter("xT", [DIM, TOK], BF, isOutput=False)
    wq = nc.declare_dram_parameter("wq", [128, NKT * 128], BF, isOutput=False)
    wk = nc.declare_dram_parameter("wk", [128, NKT * 128], BF, isOutput=False)
    wv = nc.declare_dram_parameter("wv", [128, NKT * 128], BF, isOutput=False)
    wo = nc.declare_dram_parameter("wo", [128, NKT * 128], BF, isOutput=False)
    bo = nc.declare_dram_parameter("bo", [128, 1], F32, isOutput=False)
    xresT = nc.declare_dram_parameter("xresT", [128, TOK], F32, isOutput=False)
    cosqk = nc.declare_dram_parameter("cosqk", [128, TOK], BF, isOutput=False)
    sinqk = nc.declare_dram_parameter("sinqk", [128, TOK], BF, isOutput=False)
    cosv = nc.declare_dram_parameter("cosv", [128, 64 * 64], BF, isOutput=False)
    sinv = nc.declare_dram_parameter("sinv", [128, 64 * 64], BF, isOutput=False)

    attn_outs = [nc.declare_dram_parameter(f"attn{r}", [N, N], BF, isOutput=True)
                 for r in range(NR)]
    outT = nc.declare_dram_parameter("outT", [128, TOK], F32, isOutput=True)

    with tile.TileContext(nc) as tc:
        with (
            tc.tile_pool(name="wpool", bufs=1) as wpool,
            tc.tile_pool(name="qkv", bufs=1) as qkv,
            tc.tile_pool(name="dram", bufs=1, space="DRAM") as dram,
        ):
            # persistent SBUF tensors
            qT_sb = qkv.tile([128, TOK], BF)   # [h(2) x d(64), tok]
            kT_sb = qkv.tile([128, TOK], BF)
            v_sb = qkv.tile([128, 64 * 130], BF)  # [tok%128, jt(64) x h(2) x (d(64)|one)]

            w_sb = {}
            for name, t in (("wq", wq), ("wk", wk), ("wv", wv), ("wo", wo)):
                w_sb[name] = wpool.tile([128, NKT * 128], BF, name=f"{name}_sb")
                nc.sync.dma_start(w_sb[name][:], t[:])
            bo_sb = wpool.tile([128, 1], F32)
            nc.sync.dma_start(bo_sb[:], bo[:])

            # ---------------- phase 1: projections + rope ----------------
            with (
                tc.tile_pool(name="stage", bufs=3) as stage,
                tc.tile_pool(name="ppool", bufs=2, space="PSUM") as ppool,
                tc.tile_pool(name="rope", bufs=1) as rope,
            ):
                v_raw = rope.tile([128, 64 * 128], BF)
                for ch in range(NCH):
                    xs = stage.tile([128, NKT * 512], BF, name=f"xs{ch}", tag="xs")
                    # gather [dim-in-kt(128 part), kt(8), tok(512)] from xT
                    nc.sync.dma_start(
                        xs[:],
                        xT[:].rearrange("(kt p) t -> p kt t", kt=NKT, p=128)
                             [:, :, ch * 512:(ch + 1) * 512],
                    )
                    for pname, dst in (("wq", qT_sb), ("wk", kT_sb)):
                        pj = ppool.tile([128, 512], F32, name=f"p{pname}{ch}", tag=f"p{pname}")
                        for kt in range(NKT):
                            nc.tensor.matmul(
                                pj[:],
                                w_sb[pname][:, kt * 128:(kt + 1) * 128],
                                xs[:, kt * 512:(kt + 1) * 512],
                                start=(kt == 0), stop=(kt == NKT - 1),
                            )
                        nc.vector.tensor_copy(dst[:, ch * 512:(ch + 1) * 512], pj[:])
                    # v in [tok, d] layout: lhsT = x chunk subtile, rhs = wv
                    pv = ppool.tile([128, 512], F32, name=f"pv{ch}", tag="pv")
                    for sub in range(4):
                        for kt in range(NKT):
                            nc.tensor.matmul(
                                pv[:, sub * 128:(sub + 1) * 128],
                                xs[:, kt * 512 + sub * 128: kt * 512 + (sub + 1) * 128],
                                w_sb["wv"][:, kt * 128:(kt + 1) * 128],
                                start=(kt == 0), stop=(kt == NKT - 1),
                            )
                    nc.vector.tensor_copy(v_raw[:, ch * 512:(ch + 1) * 512], pv[:])

                # ---- rope on qT/kT (partition-shift via sbuf-sbuf DMA) ----
                cq = rope.tile([128, TOK], BF)
                nc.sync.dma_start(cq[:], cosqk[:])
                sq = rope.tile([128, TOK], BF)
                nc.sync.dma_start(sq[:], sinqk[:])
                for ti, t_sb in enumerate((qT_sb, kT_sb)):
                    # rot/tmp live on the same partitions as the rope rows
                    rot = rope.tile([128, TOK], BF, name=f"rot{ti}", tag="rot")
                    tmp = rope.tile([128, TOK], BF, name=f"rtmp{ti}", tag="rtmp")
                    for half in range(2):
                        t0, t1 = half * (TOK // 2), (half + 1) * (TOK // 2)
                        for h in range(2):
                            hb = h * 64
                            # rot[0:16]=t[16:32]; rot[16:32]=t[0:16] (sign in sinqk)
                            nc.sync.dma_start(rot[hb:hb + 16, t0:t1],
                                              t_sb[hb + 16:hb + 32, t0:t1])
                            nc.sync.dma_start(rot[hb + 16:hb + 32, t0:t1],
                                              t_sb[hb:hb + 16, t0:t1])
                        for h in range(2):
                            hb = h * 64
                            nc.vector.tensor_mul(
                                tmp[hb:hb + 32, t0:t1], rot[hb:hb + 32, t0:t1],
                                sq[hb:hb + 32, t0:t1])
                            nc.vector.tensor_mul(
                                t_sb[hb:hb + 32, t0:t1], t_sb[hb:hb + 32, t0:t1],
                                cq[hb:hb + 32, t0:t1])
                            nc.vector.tensor_add(
                                t_sb[hb:hb + 32, t0:t1], t_sb[hb:hb + 32, t0:t1],
                                tmp[hb:hb + 32, t0:t1])

                # ---- rope on v (free-axis strips; layout jt x h x d) ----
                cv = rope.tile([128, 64 * 64], BF)
                nc.sync.dma_start(cv[:], cosv[:])
                sv = rope.tile([128, 64 * 64], BF)
                nc.sync.dma_start(sv[:], sinv[:])
                vtmp = rope.tile([128, 64 * 128], BF)

                def vap(t, lo, w):
                    return t[:].rearrange("p (a h d) -> p a h d", a=64, h=2, d=64)[:, :, :, lo:lo + w]

                def vsap(t, lo, w):
                    return t[:].rearrange("p (a h d) -> p a h d", a=64, h=2, d=65)[:, :, :, lo:lo + w]

                def cap(t, lo, w):
                    return t[:].rearrange("p (a h d) -> p a h d", a=64, h=2, d=32)[:, :, :, lo:lo + w]

                # strip0 (d 0:16): v = raw0*cos0 + raw1*sin0   (sin0 = -sin)
                # strip1 (d 16:32): v = raw1*cos1 + raw0*sin1  (sin1 = +sin)
                nc.vector.tensor_mul(vsap(v_sb, 0, 16), vap(v_raw, 0, 16), cap(cv, 0, 16))
                nc.vector.tensor_mul(vsap(v_sb, 16, 16), vap(v_raw, 16, 16), cap(cv, 16, 16))
                nc.vector.tensor_mul(vap(vtmp, 0, 16), vap(v_raw, 16, 16), cap(sv, 0, 16))
                nc.vector.tensor_mul(vap(vtmp, 16, 16), vap(v_raw, 0, 16), cap(sv, 16, 16))
                nc.vector.tensor_add(vsap(v_sb, 0, 32), vsap(v_sb, 0, 32), vap(vtmp, 0, 32))
                nc.vector.tensor_copy(vsap(v_sb, 32, 32), vap(v_raw, 32, 32))
                # ones column for the fused row-sum in attn@v
                nc.vector.memset(
                    v_sb[:].rearrange("p (a h d) -> p a h d", a=64, h=2, d=65)
                        [:, :, :, 64:65], 1.0)

            # ---------------- phase 2: attention rounds ----------------
            # T-major: dotsT = k q^T per j-tile; exp (unnormalized) feeds
            # attn@v directly (ones column gives row sums in pav row 64);
            # normalization = reciprocal + partition-broadcast + TT multiply;
            # attn written j-major (host transposes during unshard).
            ag_in = [dram.tile([128, N], BF, name=f"ag_in{g}") for g in range(B)]
            ag_out = [dram.tile([8 * 128, N], BF, name=f"ag_out{g}",
                                addr_space="Shared") for g in range(B)]

            with (
                tc.tile_pool(name="ring", bufs=20) as ring,
                tc.tile_pool(name="dps", bufs=2, space="PSUM") as dps,
                tc.tile_pool(name="avps", bufs=1, space="PSUM") as avps,
            ):
                for r in range(NR):
                    b, hl = r // 2, r % 2
                    hb = hl * 64
                    tok0 = b * N
                    pav = avps.tile([65, N], F32, name=f"pav{r}", tag="pav")
                    ats = []
                    for jt in range(NIT):
                        at = ring.tile([128, N], BF, name=f"at{r}_{jt}", tag="at")
                        ats.append(at)
                        for ih in range(4):
                            ps = dps.tile([128, 512], F32, name=f"psd{r}_{jt}_{ih}",
                                          tag="psd", bufs=4)
                            nc.tensor.matmul(
                                ps[:],
                                kT_sb[hb:hb + 64, tok0 + jt * 128: tok0 + (jt + 1) * 128],
                                qT_sb[hb:hb + 64, tok0 + ih * 512:
                                      tok0 + (ih + 1) * 512],
                                start=True, stop=True,
                            )
                            nc.scalar.activation(
                                at[:, ih * 512:(ih + 1) * 512], ps[:],
                                AF.Exp, bias=0.0, scale=SCALE)
                        # attn@v (+row sums via ones col): accumulate over jt
                        for sub in range(4):
                            nc.tensor.matmul(
                                pav[:, sub * 512:(sub + 1) * 512],
                                v_sb[:, (b * NIT + jt) * 130 + hl * 65:
                                     (b * NIT + jt) * 130 + (hl + 1) * 65],
                                at[:, sub * 512:(sub + 1) * 512],
                                start=(jt == 0), stop=(jt == NIT - 1),
                            )
                    # round tail: softmax scales + normalize + writes
                    inv_row = ring.tile([1, N], BF, name=f"invr{r}", tag="invrow", bufs=2)
                    with nc.allow_low_precision(reason="softmax inv scale bf16"):
                        nc.vector.reciprocal(inv_row[:], pav[64:65, :])
                    inv_b = ring.tile([128, N], BF, name=f"invb{r}", tag="invb", bufs=2)
                    nc.gpsimd.partition_broadcast(inv_b[:], inv_row[:])
                    for jt in range(NIT):
                        nc.vector.tensor_mul(ats[jt][:], ats[jt][:], inv_b[:])
                        nc.gpsimd.dma_start(
                            attn_outs[r][jt * 128:(jt + 1) * 128, :], ats[jt][:])
                    av = ring.tile([64, N], BF, name=f"av{r}", tag="av", bufs=2)
                    nc.vector.tensor_mul(av[:], pav[0:64, :], inv_b[0:64, :])
                    nc.gpsimd.dma_start(
                        ag_in[b][hl * 64:(hl + 1) * 64, :], av[:])
                    if hl == 1:
                        nc.gpsimd.collective_compute(
                            "AllGather", mybir.AluOpType.bypass,
                            replica_groups=[list(range(N_CORES))],
                            ins=[ag_in[b][:].opt()],
                            outs=[ag_out[b][:].opt()],
                        )

            # ---------------- phase 3: output projection ----------------
            with (
                tc.tile_pool(name="ostage", bufs=3) as ostage,
                tc.tile_pool(name="ops", bufs=2, space="PSUM") as ops,
            ):
                for b in range(B):
                    for ch in range(4):
                        rs = ostage.tile([128, NKT * 512], BF, name=f"rs{b}_{ch}", tag="rs")
                        nc.sync.dma_start(
                            rs[:],
                            ag_out[b][:].rearrange("(kt p) t -> p kt t", kt=NKT, p=128)
                                [:, :, ch * 512:(ch + 1) * 512],
                        )
                        po = ops.tile([128, 512], F32, name=f"po{b}_{ch}", tag="po")
                        for kt in range(NKT):
                            nc.tensor.matmul(
                                po[:],
                                w_sb["wo"][:, kt * 128:(kt + 1) * 128],
                                rs[:, kt * 512:(kt + 1) * 512],
                                start=(kt == 0), stop=(kt == NKT - 1),
                            )
                        xr = ostage.tile([128, 512], F32, name=f"xr{b}_{ch}", tag="xr")
                        nc.sync.dma_start(
                            xr[:], xresT[:, b * N + ch * 512: b * N + (ch + 1) * 512])
                        osb = ostage.tile([128, 512], F32, name=f"osb{b}_{ch}", tag="osb")
                        nc.vector.tensor_scalar_add(osb[:], po[:], bo_sb[:, 0:1])
                        nc.vector.tensor_add(osb[:], osb[:], xr[:])
                        nc.sync.dma_start(
                            outT[:, b * N + ch * 512: b * N + (ch + 1) * 512], osb[:])

    nc.compile()
    return nc


def _host_inputs(x, rotary_pos_emb, Wq, Wk, Wv, Wo, bo):
    """Build the 8 per-core input maps (host-side sharding/layout prep)."""
    bf = ml_dtypes.bfloat16
    xf = np.ascontiguousarray(x.reshape(TOK, DIM))
    xT_f32 = np.ascontiguousarray(xf.T)            # [DIM, TOK]
    xT_bf = xT_f32.astype(bf)

    # rope tables from rotary_pos_emb [1,1,N,32] = concat(f_half, f_half)
    f_half = np.asarray(rotary_pos_emb[0, 0, :, :16], dtype=np.float64)  # [N, 16]
    ch = np.cos(f_half).astype(np.float32)
    sh = np.sin(f_half).astype(np.float32)

    # qk (T layout): rows 0:16 -> cos/-sin, 16:32 -> cos/+sin, dup at rows 64..
    cos32 = np.concatenate([ch.T, ch.T], axis=0)           # [32, N]
    sin32 = np.concatenate([-sh.T, sh.T], axis=0)          # [32, N]
    cosqk = np.zeros((128, TOK), np.float32)
    sinqk = np.zeros((128, TOK), np.float32)
    for hh in range(2):
        cosqk[hh * 64:hh * 64 + 32] = np.tile(cos32, (1, B))
        sinqk[hh * 64:hh * 64 + 32] = np.tile(sin32, (1, B))
    cosqk = cosqk.astype(bf)
    sinqk = sinqk.astype(bf)

    # v (token-major layout): per flat-token-tile blocks [h(2) x d(32)]
    pos = np.arange(TOK) % N
    cblk = np.concatenate([ch, ch], axis=1)[pos]           # [TOK, 32]
    sblk = np.concatenate([-sh, sh], axis=1)[pos]          # [TOK, 32]
    cosv = np.concatenate([cblk, cblk], axis=1)            # [TOK, 64] (h dup)
    sinv = np.concatenate([sblk, sblk], axis=1)
    cosv = cosv.reshape(64, 128, 64).transpose(1, 0, 2).reshape(128, 64 * 64).astype(bf)
    sinv = sinv.reshape(64, 128, 64).transpose(1, 0, 2).reshape(128, 64 * 64).astype(bf)

    def wslice(W, c):
        wc = W[:, c * 128:(c + 1) * 128]                   # [1024, 128]
        return np.ascontiguousarray(
            wc.reshape(NKT, 128, 128).transpose(1, 0, 2).reshape(128, NKT * 128)
        ).astype(bf)

    in_maps = []
    for c in range(N_CORES):
        in_maps.append({
            "xT": xT_bf,
            "wq": wslice(np.asarray(Wq), c),
            "wk": wslice(np.asarray(Wk), c),
            "wv": wslice(np.asarray(Wv), c),
            "wo": wslice(np.asarray(Wo), c),
            "bo": np.ascontiguousarray(np.asarray(bo)[c * 128:(c + 1) * 128]
                                       ).reshape(128, 1).astype(np.float32),
            "xresT": np.ascontiguousarray(xT_f32[c * 128:(c + 1) * 128]),
            "cosqk": cosqk, "sinqk": sinqk, "cosv": cosv, "sinv": sinv,
        })
    return in_maps


def run(inputs, trace=False, trace_kwargs=None):
    """Run the SPMD kernel; returns (results_list, exec_time_ns)."""
    global _BUILT
    if _BUILT is None:
        _BUILT = build()
    in_maps = _host_inputs(**inputs)
    kw = {}
    if trace:
        kw["trace"] = True
        if trace_kwargs:
            kw.update(trace_kwargs)
    res = run_bass_kernel_spmd(_BUILT, in_maps, core_ids=list(range(N_CORES)), **kw)
    return res.results, res.exec_time_ns


def assemble_outputs(results):
    out = np.empty((B, N, DIM), np.float32)
    attn = np.empty((B, HEADS, N, N), np.float32)
    for c in range(N_CORES):
        rr = results[c]
        out.reshape(TOK, DIM)[:, c * 128:(c + 1) * 128] = rr["outT"].T
        for r in range(NR):
            b, hl = r // 2, r % 2
            attn[b, 2 * c + hl] = np.asarray(rr[f"attn{r}"]).T.astype(np.float32)
    return out, attn


def kernel(**inputs):
    results, _ = run(inputs)
    return assemble_outputs(results)


# revision 25
# speedup vs baseline: 1.0099x; 1.0099x over previous
"""Distributed Trainium2 Bass kernel for multi-head attention w/ partial RoPE.

Reference math (B=4, N=2048, DIM=1024, H=16, dh=64, rot=32):
  q,k,v = x@Wq, x@Wk, x@Wv (per-head views), partial rope on first 32 ch of
  q,k,v; attn = softmax(q k^T * dh^-0.5); out = (attn @ v) @ Wo + bo + x.
  Returns (out, attn).

Sharding: tensor-parallel over heads across 8 NeuronCores, 2 heads/core.
Per core: project full x against its 128 columns of Wq/Wk/Wv, run attention
for its 2 heads (attn slice written bf16), AllGather per-head attention
outputs (bf16, per batch), then compute a 128-column slice of the output
projection (+bias+residual).
"""
import os
import sys

sys.path.insert(0, "/opt/trn_rl_repo")


import numpy as np
import ml_dtypes

import concourse.bass as bass
import concourse.tile as tile
from concourse import bacc, mybir
from concourse.bass_utils import run_bass_kernel_spmd

BF = mybir.dt.bfloat16
F32 = mybir.dt.float32
AF = mybir.ActivationFunctionType

N_CORES = 8
B, N, DIM, HEADS, ROT = 4, 2048, 1024, 16, 32
DH = DIM // HEADS            # 64
TOK = B * N                  # 8192
SCALE = DH ** -0.5           # 0.125
NKT = DIM // 128             # 8 contraction tiles for projections
NCH = TOK // 512             # 16 token chunks for projections
NIT = N // 128               # 16 i-tiles per round
NR = 2 * B                   # 8 attention rounds (batch x local head)

_BUILT = None


def build():
    nc = bacc.Bacc("TRN2", target_bir_lowering=False, debug=False, num_devices=N_CORES)

    # ---- parameters (per-core shards prepared on host) ----
    xT = nc.declare_dram_parameter("xT", [DIM, TOK], BF, isOutput=False)
    wq = nc.declare_dram_parameter("wq", [128, NKT * 128], BF, isOutput=False)
    wk = nc.declare_dram_parameter("wk", [128, NKT * 128], BF, isOutput=False)
    wv = nc.declare_dram_parameter("wv", [128, NKT * 128], BF, isOutput=False)
    wo = nc.declare_dram_parameter("wo", [128, NKT * 128], BF, isOutput=False)
    bo = nc.declare_dram_parameter("bo", [128, 1], F32, isOutput=False)
    xresT = nc.declare_dram_parameter("xresT", [128, TOK], F32, isOutput=False)
    cosqk = nc.declare_dram_parameter("cosqk", [128, TOK], BF, isOutput=False)
    sinqk = nc.declare_dram_parameter("sinqk", [128, TOK], BF, isOutput=False)
    cosv = nc.declare_dram_parameter("cosv", [128, 64 * 64], BF, isOutput=False)
    sinv = nc.declare_dram_parameter("sinv", [128, 64 * 64], BF, isOutput=False)

    attn_outs = [nc.declare_dram_parameter(f"attn{r}", [N, N], BF, isOutput=True)
                 for r in range(NR)]
    outT = nc.declare_dram_parameter("outT", [128, TOK], F32, isOutput=True)

    with tile.TileContext(nc) as tc:
        with (
            tc.tile_pool(name="wpool", bufs=1) as wpool,
            tc.tile_pool(name="qkv", bufs=1) as qkv,
            tc.tile_pool(name="dram", bufs=1, space="DRAM") as dram,
        ):
            # persistent SBUF tensors
            qT_sb = qkv.tile([128, TOK], BF)   # [h(2) x d(64), tok]
            kT_sb = qkv.tile([128, TOK], BF)
            v_sb = qkv.tile([128, 64 * 130], BF)  # [tok%128, jt(64) x h(2) x (d(64)|one)]

            w_sb = {}
            for name, t in (("wq", wq), ("wk", wk), ("wv", wv), ("wo", wo)):
                w_sb[name] = wpool.tile([128, NKT * 128], BF, name=f"{name}_sb")
                nc.sync.dma_start(w_sb[name][:], t[:])
            bo_sb = wpool.tile([128, 1], F32)
            nc.sync.dma_start(bo_sb[:], bo[:])

            # ---------------- phase 1: projections + rope ----------------
            with (
                tc.tile_pool(name="stage", bufs=3) as stage,
                tc.tile_pool(name="ppool", bufs=2, space="PSUM") as ppool,
                tc.tile_pool(name="rope", bufs=1) as rope,
            ):
                v_raw = rope.tile([128, 64 * 128], BF)
                for ch in range(NCH):
                    xs = stage.tile([128, NKT * 512], BF, name=f"xs{ch}", tag="xs")
                    # gather [dim-in-kt(128 part), kt(8), tok(512)] from xT
                    nc.sync.dma_start(
                        xs[:],
                        xT[:].rearrange("(kt p) t -> p kt t", kt=NKT, p=128)
                             [:, :, ch * 512:(ch + 1) * 512],
                    )
                    for pname, dst in (("wq", qT_sb), ("wk", kT_sb)):
                        pj = ppool.tile([128, 512], F32, name=f"p{pname}{ch}", tag=f"p{pname}")
                        for kt in range(NKT):
                            nc.tensor.matmul(
                                pj[:],
                                w_sb[pname][:, kt * 128:(kt + 1) * 128],
                                xs[:, kt * 512:(kt + 1) * 512],
                                start=(kt == 0), stop=(kt == NKT - 1),
                            )
                        nc.vector.tensor_copy(dst[:, ch * 512:(ch + 1) * 512], pj[:])
                    # v in [tok, d] layout: lhsT = x chunk subtile, rhs = wv
                    pv = ppool.tile([128, 512], F32, name=f"pv{ch}", tag="pv")
                    for sub in range(4):
                        for kt in range(NKT):
                            nc.tensor.matmul(
                                pv[:, sub * 128:(sub + 1) * 128],
                                xs[:, kt * 512 + sub * 128: kt * 512 + (sub + 1) * 128],
                                w_sb["wv"][:, kt * 128:(kt + 1) * 128],
                                start=(kt == 0), stop=(kt == NKT - 1),
                            )
                    nc.vector.tensor_copy(v_raw[:, ch * 512:(ch + 1) * 512], pv[:])

                # ---- rope on qT/kT (partition-shift via sbuf-sbuf DMA) ----
                cq = rope.tile([128, TOK], BF)
                nc.sync.dma_start(cq[:], cosqk[:])
                sq = rope.tile([128, TOK], BF)
                nc.sync.dma_start(sq[:], sinqk[:])
                for ti, t_sb in enumerate((qT_sb, kT_sb)):
                    # rot/tmp live on the same partitions as the rope rows
                    rot = rope.tile([128, TOK], BF, name=f"rot{ti}", tag="rot")
                    tmp = rope.tile([128, TOK], BF, name=f"rtmp{ti}", tag="rtmp")
                    for half in range(2):
                        t0, t1 = half * (TOK // 2), (half + 1) * (TOK // 2)
                        for h in range(2):
                            hb = h * 64
                            # rot[0:16]=t[16:32]; rot[16:32]=t[0:16] (sign in sinqk)
                            nc.sync.dma_start(rot[hb:hb + 16, t0:t1],
                                              t_sb[hb + 16:hb + 32, t0:t1])
                            nc.sync.dma_start(rot[hb + 16:hb + 32, t0:t1],
                                              t_sb[hb:hb + 16, t0:t1])
                        for h in range(2):
                            hb = h * 64
                            nc.vector.tensor_mul(
                                tmp[hb:hb + 32, t0:t1], rot[hb:hb + 32, t0:t1],
                                sq[hb:hb + 32, t0:t1])
                            nc.vector.tensor_mul(
                                t_sb[hb:hb + 32, t0:t1], t_sb[hb:hb + 32, t0:t1],
                                cq[hb:hb + 32, t0:t1])
                            nc.vector.tensor_add(
                                t_sb[hb:hb + 32, t0:t1], t_sb[hb:hb + 32, t0:t1],
                                tmp[hb:hb + 32, t0:t1])

                # ---- rope on v (free-axis strips; layout jt x h x d) ----
                cv = rope.tile([128, 64 * 64], BF)
                nc.sync.dma_start(cv[:], cosv[:])
                sv = rope.tile([128, 64 * 64], BF)
                nc.sync.dma_start(sv[:], sinv[:])
                vtmp = rope.tile([128, 64 * 128], BF)

                def vap(t, lo, w):
                    return t[:].rearrange("p (a h d) -> p a h d", a=64, h=2, d=64)[:, :, :, lo:lo + w]

                def vsap(t, lo, w):
                    return t[:].rearrange("p (a h d) -> p a h d", a=64, h=2, d=65)[:, :, :, lo:lo + w]

                def cap(t, lo, w):
                    return t[:].rearrange("p (a h d) -> p a h d", a=64, h=2, d=32)[:, :, :, lo:lo + w]

                # strip0 (d 0:16): v = raw0*cos0 + raw1*sin0   (sin0 = -sin)
                # strip1 (d 16:32): v = raw1*cos1 + raw0*sin1  (sin1 = +sin)
                nc.vector.tensor_mul(vsap(v_sb, 0, 16), vap(v_raw, 0, 16), cap(cv, 0, 16))
                nc.vector.tensor_mul(vsap(v_sb, 16, 16), vap(v_raw, 16, 16), cap(cv, 16, 16))
                nc.vector.tensor_mul(vap(vtmp, 0, 16), vap(v_raw, 16, 16), cap(sv, 0, 16))
                nc.vector.tensor_mul(vap(vtmp, 16, 16), vap(v_raw, 0, 16), cap(sv, 16, 16))
                nc.vector.tensor_add(vsap(v_sb, 0, 32), vsap(v_sb, 0, 32), vap(vtmp, 0, 32))
                nc.vector.tensor_copy(vsap(v_sb, 32, 32), vap(v_raw, 32, 32))
                # ones column for the fused row-sum in attn@v
                nc.vector.memset(
                    v_sb[:].rearrange("p (a h d) -> p a h d", a=64, h=2, d=65)
                        [:, :, :, 64:65], 1.0)

            # ---------------- phase 2: attention rounds ----------------
            # T-major: dotsT = k q^T per j-tile; exp (unnormalized) feeds
            # attn@v directly (ones column gives row sums in pav row 64);
            # normalization = reciprocal + partition-broadcast + TT multiply;
            # attn written j-major (host transposes during unshard).
            ag_in = [dram.tile([128, N], BF, name=f"ag_in{g}") for g in range(B)]
            ag_out = [dram.tile([8 * 128, N], BF, name=f"ag_out{g}",
                                addr_space="Shared") for g in range(B)]

            with (
                tc.tile_pool(name="ring", bufs=20) as ring,
                tc.tile_pool(name="dps", bufs=2, space="PSUM") as dps,
                tc.tile_pool(name="avps", bufs=1, space="PSUM") as avps,
            ):
                for r in range(NR):
                    b, hl = r // 2, r % 2
                    hb = hl * 64
                    tok0 = b * N
                    pav = avps.tile([65, N], F32, name=f"pav{r}", tag="pav")
                    ats = []
                    for jt in range(NIT):
                        at = ring.tile([128, N], BF, name=f"at{r}_{jt}", tag="at")
                        ats.append(at)
                        for ih in range(2):
                            ps = dps.tile([128, 1024], F32, name=f"psd{r}_{jt}_{ih}",
                                          tag="psd")
                            for iq in range(2):
                                nc.tensor.matmul(
                                    ps[:, iq * 512:(iq + 1) * 512],
                                    kT_sb[hb:hb + 64, tok0 + jt * 128: tok0 + (jt + 1) * 128],
                                    qT_sb[hb:hb + 64, tok0 + ih * 1024 + iq * 512:
                                          tok0 + ih * 1024 + (iq + 1) * 512],
                                    start=True, stop=True,
                                )
                            nc.scalar.activation(
                                at[:, ih * 1024:(ih + 1) * 1024], ps[:],
                                AF.Exp, bias=0.0, scale=SCALE)
                        # attn@v (+row sums via ones col): accumulate over jt
                        for sub in range(4):
                            nc.tensor.matmul(
                                pav[:, sub * 512:(sub + 1) * 512],
                                v_sb[:, (b * NIT + jt) * 130 + hl * 65:
                                     (b * NIT + jt) * 130 + (hl + 1) * 65],
                                at[:, sub * 512:(sub + 1) * 512],
                                start=(jt == 0), stop=(jt == NIT - 1),
                            )
                    # round tail: softmax scales + normalize + writes
                    inv_row = ring.tile([1, N], BF, name=f"invr{r}", tag="invrow", bufs=2)
                    with nc.allow_low_precision(reason="softmax inv scale bf16"):
                        nc.vector.reciprocal(inv_row[:], pav[64:65, :])
                    inv_b = ring.tile([128, N], BF, name=f"invb{r}", tag="invb", bufs=2)
                    nc.gpsimd.partition_broadcast(inv_b[:], inv_row[:])
                    for jt in range(NIT):
                        nc.vector.tensor_mul(ats[jt][:], ats[jt][:], inv_b[:])
                        nc.sync.dma_start(
                            attn_outs[r][jt * 128:(jt + 1) * 128, :], ats[jt][:])
                    av = ring.tile([64, N], BF, name=f"av{r}", tag="av", bufs=2)
                    nc.vector.tensor_mul(av[:], pav[0:64, :], inv_b[0:64, :])
                    nc.gpsimd.dma_start(
                        ag_in[b][hl * 64:(hl + 1) * 64, :], av[:])
                    if hl == 1:
                        nc.gpsimd.collective_compute(
                            "AllGather", mybir.AluOpType.bypass,
                            replica_groups=[list(range(N_CORES))],
                            ins=[ag_in[b][:].opt()],
                            outs=[ag_out[b][:].opt()],
                        )

            # ---------------- phase 3: output projection ----------------
            with (
                tc.tile_pool(name="ostage", bufs=3) as ostage,
                tc.tile_pool(name="ops", bufs=2, space="PSUM") as ops,
            ):
                for b in range(B):
                    for ch in range(4):
                        rs = ostage.tile([128, NKT * 512], BF, name=f"rs{b}_{ch}", tag="rs")
                        nc.sync.dma_start(
                            rs[:],
                            ag_out[b][:].rearrange("(kt p) t -> p kt t", kt=NKT, p=128)
                                [:, :, ch * 512:(ch + 1) * 512],
                        )
                        po = ops.tile([128, 512], F32, name=f"po{b}_{ch}", tag="po")
                        for kt in range(NKT):
                            nc.tensor.matmul(
                                po[:],
                                w_sb["wo"][:, kt * 128:(kt + 1) * 128],
                                rs[:, kt * 512:(kt + 1) * 512],
                                start=(kt == 0), stop=(kt == NKT - 1),
                            )
                        xr = ostage.tile([128, 512], F32, name=f"xr{b}_{ch}", tag="xr")
                        nc.sync.dma_start(
                            xr[:], xresT[:, b * N + ch * 512: b * N + (ch + 1) * 512])
                        osb = ostage.tile([128, 512], F32, name=f"osb{b}_{ch}", tag="osb")
                        nc.vector.tensor_scalar_add(osb[:], po[:], bo_sb[:, 0:1])
                        nc.vector.tensor_add(osb[:], osb[:], xr[:])
                        nc.sync.dma_start(
                            outT[:, b * N + ch * 512: b * N + (ch + 1) * 512], osb[:])

    nc.compile()
    return nc


def _host_inputs(x, rotary_pos_emb, Wq, Wk, Wv, Wo, bo):
    """Build the 8 per-core input maps (host-side sharding/layout prep)."""
    bf = ml_dtypes.bfloat16
    xf = np.ascontiguousarray(x.reshape(TOK, DIM))
    xT_f32 = np.ascontiguousarray(xf.T)            # [DIM, TOK]
    xT_bf = xT_f32.astype(bf)

    # rope tables from rotary_pos_emb [1,1,N,32] = concat(f_half, f_half)
    f_half = np.asarray(rotary_pos_emb[0, 0, :, :16], dtype=np.float64)  # [N, 16]
    ch = np.cos(f_half).astype(np.float32)
    sh = np.sin(f_half).astype(np.float32)

    # qk (T layout): rows 0:16 -> cos/-sin, 16:32 -> cos/+sin, dup at rows 64..
    cos32 = np.concatenate([ch.T, ch.T], axis=0)           # [32, N]
    sin32 = np.concatenate([-sh.T, sh.T], axis=0)          # [32, N]
    cosqk = np.zeros((128, TOK), np.float32)
    sinqk = np.zeros((128, TOK), np.float32)
    for hh in range(2):
        cosqk[hh * 64:hh * 64 + 32] = np.tile(cos32, (1, B))
        sinqk[hh * 64:hh * 64 + 32] = np.tile(sin32, (1, B))
    cosqk = cosqk.astype(bf)
    sinqk = sinqk.astype(bf)

    # v (token-major layout): per flat-token-tile blocks [h(2) x d(32)]
    pos = np.arange(TOK) % N
    cblk = np.concatenate([ch, ch], axis=1)[pos]           # [TOK, 32]
    sblk = np.concatenate([-sh, sh], axis=1)[pos]          # [TOK, 32]
    cosv = np.concatenate([cblk, cblk], axis=1)            # [TOK, 64] (h dup)
    sinv = np.concatenate([sblk, sblk], axis=1)
    cosv = cosv.reshape(64, 128, 64).transpose(1, 0, 2).reshape(128, 64 * 64).astype(bf)
    sinv = sinv.reshape(64, 128, 64).transpose(1, 0, 2).reshape(128, 64 * 64).astype(bf)

    def wslice(W, c):
        wc = W[:, c * 128:(c + 1) * 128]                   # [1024, 128]
        return np.ascontiguousarray(
            wc.reshape(NKT, 128, 128).transpose(1, 0, 2).reshape(128, NKT * 128)
        ).astype(bf)

    in_maps = []
    for c in range(N_CORES):
        in_maps.append({
            "xT": xT_bf,
            "wq": wslice(np.asarray(Wq), c),
            "wk": wslice(np.asarray(Wk), c),
            "wv": wslice(np.asarray(Wv), c),
            "wo": wslice(np.asarray(Wo), c),
            "bo": np.ascontiguousarray(np.asarray(bo)[c * 128:(c + 1) * 128]
                                       ).reshape(128, 1).astype(np.float32),
            "xresT": np.ascontiguousarray(xT_f32[c * 128:(c + 1) * 128]),
            "cosqk": cosqk, "sinqk": sinqk, "cosv": cosv, "sinv": sinv,
        })
    return in_maps


def run(inputs, trace=False, trace_kwargs=None):
    """Run the SPMD kernel; returns (results_list, exec_time_ns)."""
    global _BUILT
    if _BUILT is None:
        _BUILT = build()
    in_maps = _host_inputs(**inputs)
    kw = {}
    if trace:
        kw["trace"] = True
        if trace_kwargs:
            kw.update(trace_kwargs)
    res = run_bass_kernel_spmd(_BUILT, in_maps, core_ids=list(range(N_CORES)), **kw)
    return res.results, res.exec_time_ns


def assemble_outputs(results):
    out = np.empty((B, N, DIM), np.float32)
    attn = np.empty((B, HEADS, N, N), np.float32)
    for c in range(N_CORES):
        rr = results[c]
        out.reshape(TOK, DIM)[:, c * 128:(c + 1) * 128] = rr["outT"].T
        for r in range(NR):
            b, hl = r // 2, r % 2
            attn[b, 2 * c + hl] = np.asarray(rr[f"attn{r}"]).T.astype(np.float32)
    return out, attn


def kernel(**inputs):
    results, _ = run(inputs)
    return assemble_outputs(results)


# revision 26
# speedup vs baseline: 1.0345x; 1.0243x over previous
"""Distributed Trainium2 Bass kernel for multi-head attention w/ partial RoPE.

Reference math (B=4, N=2048, DIM=1024, H=16, dh=64, rot=32):
  q,k,v = x@Wq, x@Wk, x@Wv (per-head views), partial rope on first 32 ch of
  q,k,v; attn = softmax(q k^T * dh^-0.5); out = (attn @ v) @ Wo + bo + x.
  Returns (out, attn).

Sharding: tensor-parallel over heads across 8 NeuronCores, 2 heads/core.
Per core: project full x against its 128 columns of Wq/Wk/Wv, run attention
for its 2 heads (attn slice written bf16), AllGather per-head attention
outputs (bf16, per batch), then compute a 128-column slice of the output
projection (+bias+residual).
"""
import os
import sys

sys.path.insert(0, "/opt/trn_rl_repo")


import numpy as np
import ml_dtypes

import concourse.bass as bass
import concourse.tile as tile
from concourse import bacc, mybir
from concourse.bass_utils import run_bass_kernel_spmd

BF = mybir.dt.bfloat16
F32 = mybir.dt.float32
AF = mybir.ActivationFunctionType

N_CORES = 8
B, N, DIM, HEADS, ROT = 4, 2048, 1024, 16, 32
DH = DIM // HEADS            # 64
TOK = B * N                  # 8192
SCALE = DH ** -0.5           # 0.125
NKT = DIM // 128             # 8 contraction tiles for projections
NCH = TOK // 512             # 16 token chunks for projections
NIT = N // 128               # 16 i-tiles per round
NR = 2 * B                   # 8 attention rounds (batch x local head)

_BUILT = None


def build():
    nc = bacc.Bacc("TRN2", target_bir_lowering=False, debug=False, num_devices=N_CORES)

    # ---- parameters (per-core shards prepared on host) ----
    xT = nc.declare_dram_parameter("xT", [DIM, TOK], BF, isOutput=False)
    wq = nc.declare_dram_parameter("wq", [128, NKT * 128], BF, isOutput=False)
    wk = nc.declare_dram_parameter("wk", [128, NKT * 128], BF, isOutput=False)
    wv = nc.declare_dram_parameter("wv", [128, NKT * 128], BF, isOutput=False)
    wo = nc.declare_dram_parameter("wo", [128, NKT * 128], BF, isOutput=False)
    bo = nc.declare_dram_parameter("bo", [128, 1], F32, isOutput=False)
    xresT = nc.declare_dram_parameter("xresT", [128, TOK], F32, isOutput=False)
    cosqk = nc.declare_dram_parameter("cosqk", [128, TOK], BF, isOutput=False)
    sinqk = nc.declare_dram_parameter("sinqk", [128, TOK], BF, isOutput=False)
    cosv = nc.declare_dram_parameter("cosv", [128, 64 * 64], BF, isOutput=False)
    sinv = nc.declare_dram_parameter("sinv", [128, 64 * 64], BF, isOutput=False)

    attn_outs = [nc.declare_dram_parameter(f"attn{r}", [N, N], BF, isOutput=True)
                 for r in range(NR)]
    outT = nc.declare_dram_parameter("outT", [128, TOK], F32, isOutput=True)

    with tile.TileContext(nc) as tc:
        with (
            tc.tile_pool(name="wpool", bufs=1) as wpool,
            tc.tile_pool(name="qkv", bufs=1) as qkv,
            tc.tile_pool(name="dram", bufs=1, space="DRAM") as dram,
        ):
            # persistent SBUF tensors
            qT_sb = qkv.tile([128, TOK], BF)   # [h(2) x d(64), tok]
            kT_sb = qkv.tile([128, TOK], BF)
            v_sb = qkv.tile([128, 64 * 130], BF)  # [tok%128, jt(64) x h(2) x (d(64)|one)]

            w_sb = {}
            for name, t in (("wq", wq), ("wk", wk), ("wv", wv), ("wo", wo)):
                w_sb[name] = wpool.tile([128, NKT * 128], BF, name=f"{name}_sb")
                nc.sync.dma_start(w_sb[name][:], t[:])
            bo_sb = wpool.tile([128, 1], F32)
            nc.sync.dma_start(bo_sb[:], bo[:])

            # ---------------- phase 1: projections + rope ----------------
            with (
                tc.tile_pool(name="stage", bufs=3) as stage,
                tc.tile_pool(name="ppool", bufs=2, space="PSUM") as ppool,
                tc.tile_pool(name="rope", bufs=1) as rope,
            ):
                v_raw = rope.tile([128, 64 * 128], BF)
                for ch in range(NCH):
                    xs = stage.tile([128, NKT * 512], BF, name=f"xs{ch}", tag="xs")
                    # gather [dim-in-kt(128 part), kt(8), tok(512)] from xT
                    nc.sync.dma_start(
                        xs[:],
                        xT[:].rearrange("(kt p) t -> p kt t", kt=NKT, p=128)
                             [:, :, ch * 512:(ch + 1) * 512],
                    )
                    for pname, dst in (("wq", qT_sb), ("wk", kT_sb)):
                        pj = ppool.tile([128, 512], F32, name=f"p{pname}{ch}", tag=f"p{pname}")
                        for kt in range(NKT):
                            nc.tensor.matmul(
                                pj[:],
                                w_sb[pname][:, kt * 128:(kt + 1) * 128],
                                xs[:, kt * 512:(kt + 1) * 512],
                                start=(kt == 0), stop=(kt == NKT - 1),
                            )
                        nc.vector.tensor_copy(dst[:, ch * 512:(ch + 1) * 512], pj[:])
                    # v in [tok, d] layout: lhsT = x chunk subtile, rhs = wv
                    pv = ppool.tile([128, 512], F32, name=f"pv{ch}", tag="pv")
                    for sub in range(4):
                        for kt in range(NKT):
                            nc.tensor.matmul(
                                pv[:, sub * 128:(sub + 1) * 128],
                                xs[:, kt * 512 + sub * 128: kt * 512 + (sub + 1) * 128],
                                w_sb["wv"][:, kt * 128:(kt + 1) * 128],
                                start=(kt == 0), stop=(kt == NKT - 1),
                            )
                    nc.vector.tensor_copy(v_raw[:, ch * 512:(ch + 1) * 512], pv[:])

                # ---- rope on qT/kT (partition-shift via sbuf-sbuf DMA) ----
                cq = rope.tile([128, TOK], BF)
                nc.sync.dma_start(cq[:], cosqk[:])
                sq = rope.tile([128, TOK], BF)
                nc.sync.dma_start(sq[:], sinqk[:])
                for ti, t_sb in enumerate((qT_sb, kT_sb)):
                    # rot/tmp live on the same partitions as the rope rows
                    rot = rope.tile([128, TOK], BF, name=f"rot{ti}", tag="rot")
                    tmp = rope.tile([128, TOK], BF, name=f"rtmp{ti}", tag="rtmp")
                    for half in range(2):
                        t0, t1 = half * (TOK // 2), (half + 1) * (TOK // 2)
                        for h in range(2):
                            hb = h * 64
                            # rot[0:16]=t[16:32]; rot[16:32]=t[0:16] (sign in sinqk)
                            nc.sync.dma_start(rot[hb:hb + 16, t0:t1],
                                              t_sb[hb + 16:hb + 32, t0:t1])
                            nc.sync.dma_start(rot[hb + 16:hb + 32, t0:t1],
                                              t_sb[hb:hb + 16, t0:t1])
                        for h in range(2):
                            hb = h * 64
                            nc.vector.tensor_mul(
                                tmp[hb:hb + 32, t0:t1], rot[hb:hb + 32, t0:t1],
                                sq[hb:hb + 32, t0:t1])
                            nc.vector.tensor_mul(
                                t_sb[hb:hb + 32, t0:t1], t_sb[hb:hb + 32, t0:t1],
                                cq[hb:hb + 32, t0:t1])
                            nc.vector.tensor_add(
                                t_sb[hb:hb + 32, t0:t1], t_sb[hb:hb + 32, t0:t1],
                                tmp[hb:hb + 32, t0:t1])

                # ---- rope on v (free-axis strips; layout jt x h x d) ----
                cv = rope.tile([128, 64 * 64], BF)
                nc.sync.dma_start(cv[:], cosv[:])
                sv = rope.tile([128, 64 * 64], BF)
                nc.sync.dma_start(sv[:], sinv[:])
                vtmp = rope.tile([128, 64 * 128], BF)

                def vap(t, lo, w):
                    return t[:].rearrange("p (a h d) -> p a h d", a=64, h=2, d=64)[:, :, :, lo:lo + w]

                def vsap(t, lo, w):
                    return t[:].rearrange("p (a h d) -> p a h d", a=64, h=2, d=65)[:, :, :, lo:lo + w]

                def cap(t, lo, w):
                    return t[:].rearrange("p (a h d) -> p a h d", a=64, h=2, d=32)[:, :, :, lo:lo + w]

                # strip0 (d 0:16): v = raw0*cos0 + raw1*sin0   (sin0 = -sin)
                # strip1 (d 16:32): v = raw1*cos1 + raw0*sin1  (sin1 = +sin)
                nc.vector.tensor_mul(vsap(v_sb, 0, 16), vap(v_raw, 0, 16), cap(cv, 0, 16))
                nc.vector.tensor_mul(vsap(v_sb, 16, 16), vap(v_raw, 16, 16), cap(cv, 16, 16))
                nc.vector.tensor_mul(vap(vtmp, 0, 16), vap(v_raw, 16, 16), cap(sv, 0, 16))
                nc.vector.tensor_mul(vap(vtmp, 16, 16), vap(v_raw, 0, 16), cap(sv, 16, 16))
                nc.vector.tensor_add(vsap(v_sb, 0, 32), vsap(v_sb, 0, 32), vap(vtmp, 0, 32))
                nc.vector.tensor_copy(vsap(v_sb, 32, 32), vap(v_raw, 32, 32))
                # ones column for the fused row-sum in attn@v
                nc.vector.memset(
                    v_sb[:].rearrange("p (a h d) -> p a h d", a=64, h=2, d=65)
                        [:, :, :, 64:65], 1.0)

            # ---------------- phase 2: attention rounds ----------------
            # T-major: dotsT = k q^T per j-tile; exp (unnormalized) feeds
            # attn@v directly (ones column gives row sums in pav row 64);
            # normalization = reciprocal + partition-broadcast + TT multiply;
            # attn written j-major (host transposes during unshard).
            ag_in = [dram.tile([128, N], BF, name=f"ag_in{g}") for g in range(B)]
            ag_out = [dram.tile([8 * 128, N], BF, name=f"ag_out{g}",
                                addr_space="Shared") for g in range(B)]

            with (
                tc.tile_pool(name="ring", bufs=24) as ring,
                tc.tile_pool(name="dps", bufs=2, space="PSUM") as dps,
                tc.tile_pool(name="avps", bufs=1, space="PSUM") as avps,
            ):
                for r in range(NR):
                    b, hl = r // 2, r % 2
                    hb = hl * 64
                    tok0 = b * N
                    pav = avps.tile([65, N], F32, name=f"pav{r}", tag="pav")
                    ats = []
                    for jt in range(NIT):
                        at = ring.tile([128, N], BF, name=f"at{r}_{jt}", tag="at")
                        ats.append(at)
                        for ih in range(2):
                            ps = dps.tile([128, 1024], F32, name=f"psd{r}_{jt}_{ih}",
                                          tag="psd")
                            for iq in range(2):
                                nc.tensor.matmul(
                                    ps[:, iq * 512:(iq + 1) * 512],
                                    kT_sb[hb:hb + 64, tok0 + jt * 128: tok0 + (jt + 1) * 128],
                                    qT_sb[hb:hb + 64, tok0 + ih * 1024 + iq * 512:
                                          tok0 + ih * 1024 + (iq + 1) * 512],
                                    start=True, stop=True,
                                )
                            nc.scalar.activation(
                                at[:, ih * 1024:(ih + 1) * 1024], ps[:],
                                AF.Exp, bias=0.0, scale=SCALE)
                        # attn@v (+row sums via ones col): accumulate over jt
                        for sub in range(4):
                            nc.tensor.matmul(
                                pav[:, sub * 512:(sub + 1) * 512],
                                v_sb[:, (b * NIT + jt) * 130 + hl * 65:
                                     (b * NIT + jt) * 130 + (hl + 1) * 65],
                                at[:, sub * 512:(sub + 1) * 512],
                                start=(jt == 0), stop=(jt == NIT - 1),
                            )
                    # round tail: softmax scales + normalize + writes
                    inv_row = ring.tile([1, N], BF, name=f"invr{r}", tag="invrow", bufs=4)
                    with nc.allow_low_precision(reason="softmax inv scale bf16"):
                        nc.vector.reciprocal(inv_row[:], pav[64:65, :])
                    inv_b = ring.tile([128, N], BF, name=f"invb{r}", tag="invb", bufs=4)
                    nc.gpsimd.partition_broadcast(inv_b[:], inv_row[:])
                    for jt in range(NIT):
                        nc.vector.tensor_mul(ats[jt][:], ats[jt][:], inv_b[:])
                        nc.sync.dma_start(
                            attn_outs[r][jt * 128:(jt + 1) * 128, :], ats[jt][:])
                    av = ring.tile([64, N], BF, name=f"av{r}", tag="av", bufs=4)
                    nc.vector.tensor_mul(av[:], pav[0:64, :], inv_b[0:64, :])
                    nc.gpsimd.dma_start(
                        ag_in[b][hl * 64:(hl + 1) * 64, :], av[:])
                    if hl == 1:
                        nc.gpsimd.collective_compute(
                            "AllGather", mybir.AluOpType.bypass,
                            replica_groups=[list(range(N_CORES))],
                            ins=[ag_in[b][:].opt()],
                            outs=[ag_out[b][:].opt()],
                        )

            # ---------------- phase 3: output projection ----------------
            with (
                tc.tile_pool(name="ostage", bufs=3) as ostage,
                tc.tile_pool(name="ops", bufs=2, space="PSUM") as ops,
            ):
                for b in range(B):
                    for ch in range(4):
                        rs = ostage.tile([128, NKT * 512], BF, name=f"rs{b}_{ch}", tag="rs")
                        nc.sync.dma_start(
                            rs[:],
                            ag_out[b][:].rearrange("(kt p) t -> p kt t", kt=NKT, p=128)
                                [:, :, ch * 512:(ch + 1) * 512],
                        )
                        po = ops.tile([128, 512], F32, name=f"po{b}_{ch}", tag="po")
                        for kt in range(NKT):
                            nc.tensor.matmul(
                                po[:],
                                w_sb["wo"][:, kt * 128:(kt + 1) * 128],
                                rs[:, kt * 512:(kt + 1) * 512],
                                start=(kt == 0), stop=(kt == NKT - 1),
                            )
                        xr = ostage.tile([128, 512], F32, name=f"xr{b}_{ch}", tag="xr")
                        nc.sync.dma_start(
                            xr[:], xresT[:, b * N + ch * 512: b * N + (ch + 1) * 512])
                        osb = ostage.tile([128, 512], F32, name=f"osb{b}_{ch}", tag="osb")
                        nc.vector.tensor_scalar_add(osb[:], po[:], bo_sb[:, 0:1])
                        nc.vector.tensor_add(osb[:], osb[:], xr[:])
                        nc.sync.dma_start(
                            outT[:, b * N + ch * 512: b * N + (ch + 1) * 512], osb[:])

    nc.compile()
    return nc


def _host_inputs(x, rotary_pos_emb, Wq, Wk, Wv, Wo, bo):
    """Build the 8 per-core input maps (host-side sharding/layout prep)."""
    bf = ml_dtypes.bfloat16
    xf = np.ascontiguousarray(x.reshape(TOK, DIM))
    xT_f32 = np.ascontiguousarray(xf.T)            # [DIM, TOK]
    xT_bf = xT_f32.astype(bf)

    # rope tables from rotary_pos_emb [1,1,N,32] = concat(f_half, f_half)
    f_half = np.asarray(rotary_pos_emb[0, 0, :, :16], dtype=np.float64)  # [N, 16]
    ch = np.cos(f_half).astype(np.float32)
    sh = np.sin(f_half).astype(np.float32)

    # qk (T layout): rows 0:16 -> cos/-sin, 16:32 -> cos/+sin, dup at rows 64..
    cos32 = np.concatenate([ch.T, ch.T], axis=0)           # [32, N]
    sin32 = np.concatenate([-sh.T, sh.T], axis=0)          # [32, N]
    cosqk = np.zeros((128, TOK), np.float32)
    sinqk = np.zeros((128, TOK), np.float32)
    for hh in range(2):
        cosqk[hh * 64:hh * 64 + 32] = np.tile(cos32, (1, B))
        sinqk[hh * 64:hh * 64 + 32] = np.tile(sin32, (1, B))
    cosqk = cosqk.astype(bf)
    sinqk = sinqk.astype(bf)

    # v (token-major layout): per flat-token-tile blocks [h(2) x d(32)]
    pos = np.arange(TOK) % N
    cblk = np.concatenate([ch, ch], axis=1)[pos]           # [TOK, 32]
    sblk = np.concatenate([-sh, sh], axis=1)[pos]          # [TOK, 32]
    cosv = np.concatenate([cblk, cblk], axis=1)            # [TOK, 64] (h dup)
    sinv = np.concatenate([sblk, sblk], axis=1)
    cosv = cosv.reshape(64, 128, 64).transpose(1, 0, 2).reshape(128, 64 * 64).astype(bf)
    sinv = sinv.reshape(64, 128, 64).transpose(1, 0, 2).reshape(128, 64 * 64).astype(bf)

    def wslice(W, c):
        wc = W[:, c * 128:(c + 1) * 128]                   # [1024, 128]
        return np.ascontiguousarray(
            wc.reshape(NKT, 128, 128).transpose(1, 0, 2).reshape(128, NKT * 128)
        ).astype(bf)

    in_maps = []
    for c in range(N_CORES):
        in_maps.append({
            "xT": xT_bf,
            "wq": wslice(np.asarray(Wq), c),
            "wk": wslice(np.asarray(Wk), c),
            "wv": wslice(np.asarray(Wv), c),
            "wo": wslice(np.asarray(Wo), c),
            "bo": np.ascontiguousarray(np.asarray(bo)[c * 128:(c + 1) * 128]
                                       ).reshape(128, 1).astype(np.float32),
            "xresT": np.ascontiguousarray(xT_f32[c * 128:(c + 1) * 128]),
            "cosqk": cosqk, "sinqk": sinqk, "cosv": cosv, "sinv": sinv,
        })
    return in_maps


def run(inputs, trace=False, trace_kwargs=None):
    """Run the SPMD kernel; returns (results_list, exec_time_ns)."""
    global _BUILT
    if _BUILT is None:
        _BUILT = build()
    in_maps = _host_inputs(**inputs)
    kw = {}
    if trace:
        kw["trace"] = True
        if trace_kwargs:
            kw.update(trace_kwargs)
    res = run_bass_kernel_spmd(_BUILT, in_maps, core_ids=list(range(N_CORES)), **kw)
    return res.results, res.exec_time_ns


def assemble_outputs(results):
    out = np.empty((B, N, DIM), np.float32)
    attn = np.empty((B, HEADS, N, N), np.float32)
    for c in range(N_CORES):
        rr = results[c]
        out.reshape(TOK, DIM)[:, c * 128:(c + 1) * 128] = rr["outT"].T
        for r in range(NR):
            b, hl = r // 2, r % 2
            attn[b, 2 * c + hl] = np.asarray(rr[f"attn{r}"]).T.astype(np.float32)
    return out, attn


def kernel(**inputs):
    results, _ = run(inputs)
    return assemble_outputs(results)


# revision 28
# speedup vs baseline: 1.0479x; 1.0130x over previous
"""Distributed Trainium2 Bass kernel for multi-head attention w/ partial RoPE.

Reference math (B=4, N=2048, DIM=1024, H=16, dh=64, rot=32):
  q,k,v = x@Wq, x@Wk, x@Wv (per-head views), partial rope on first 32 ch of
  q,k,v; attn = softmax(q k^T * dh^-0.5); out = (attn @ v) @ Wo + bo + x.
  Returns (out, attn).

Sharding: tensor-parallel over heads across 8 NeuronCores, 2 heads/core.
Per core: project full x against its 128 columns of Wq/Wk/Wv, run attention
for its 2 heads (attn slice written bf16), AllGather per-head attention
outputs (bf16, per batch), then compute a 128-column slice of the output
projection (+bias+residual).
"""
import os
import sys

sys.path.insert(0, "/opt/trn_rl_repo")


import numpy as np
import ml_dtypes

import concourse.bass as bass
import concourse.tile as tile
from concourse import bacc, mybir
from concourse.bass_utils import run_bass_kernel_spmd

BF = mybir.dt.bfloat16
F32 = mybir.dt.float32
AF = mybir.ActivationFunctionType

N_CORES = 8
B, N, DIM, HEADS, ROT = 4, 2048, 1024, 16, 32
DH = DIM // HEADS            # 64
TOK = B * N                  # 8192
SCALE = DH ** -0.5           # 0.125
NKT = DIM // 128             # 8 contraction tiles for projections
NCH = TOK // 512             # 16 token chunks for projections
NIT = N // 128               # 16 i-tiles per round
NR = 2 * B                   # 8 attention rounds (batch x local head)

_BUILT = None


def build():
    nc = bacc.Bacc("TRN2", target_bir_lowering=False, debug=False, num_devices=N_CORES)

    # ---- parameters (per-core shards prepared on host) ----
    xT = nc.declare_dram_parameter("xT", [DIM, TOK], BF, isOutput=False)
    wq = nc.declare_dram_parameter("wq", [128, NKT * 128], BF, isOutput=False)
    wk = nc.declare_dram_parameter("wk", [128, NKT * 128], BF, isOutput=False)
    wv = nc.declare_dram_parameter("wv", [128, NKT * 128], BF, isOutput=False)
    wo = nc.declare_dram_parameter("wo", [128, NKT * 128], BF, isOutput=False)
    bo = nc.declare_dram_parameter("bo", [128, 1], F32, isOutput=False)
    xresT = nc.declare_dram_parameter("xresT", [128, TOK], F32, isOutput=False)
    cosqk = nc.declare_dram_parameter("cosqk", [128, TOK], BF, isOutput=False)
    sinqk = nc.declare_dram_parameter("sinqk", [128, TOK], BF, isOutput=False)
    cosv = nc.declare_dram_parameter("cosv", [128, 64 * 64], BF, isOutput=False)
    sinv = nc.declare_dram_parameter("sinv", [128, 64 * 64], BF, isOutput=False)

    attn_outs = [nc.declare_dram_parameter(f"attn{r}", [N, N], BF, isOutput=True)
                 for r in range(NR)]
    outT = nc.declare_dram_parameter("outT", [128, TOK], F32, isOutput=True)

    with tile.TileContext(nc) as tc:
        with (
            tc.tile_pool(name="wpool", bufs=1) as wpool,
            tc.tile_pool(name="qkv", bufs=1) as qkv,
            tc.tile_pool(name="dram", bufs=1, space="DRAM") as dram,
        ):
            # persistent SBUF tensors
            qT_sb = qkv.tile([128, TOK], BF)   # [h(2) x d(64), tok]
            kT_sb = qkv.tile([128, TOK], BF)
            v_sb = qkv.tile([128, 64 * 130], BF)  # [tok%128, jt(64) x h(2) x (d(64)|one)]

            w_sb = {}
            for name, t in (("wq", wq), ("wk", wk), ("wv", wv), ("wo", wo)):
                w_sb[name] = wpool.tile([128, NKT * 128], BF, name=f"{name}_sb")
                nc.sync.dma_start(w_sb[name][:], t[:])
            bo_sb = wpool.tile([128, 1], F32)
            nc.sync.dma_start(bo_sb[:], bo[:])

            # ---------------- phase 1: projections + rope ----------------
            with (
                tc.tile_pool(name="stage", bufs=3) as stage,
                tc.tile_pool(name="ppool", bufs=2, space="PSUM") as ppool,
                tc.tile_pool(name="rope", bufs=1) as rope,
            ):
                v_raw = rope.tile([128, 64 * 128], BF)
                for ch in range(NCH):
                    xs = stage.tile([128, NKT * 512], BF, name=f"xs{ch}", tag="xs")
                    # gather [dim-in-kt(128 part), kt(8), tok(512)] from xT
                    nc.sync.dma_start(
                        xs[:],
                        xT[:].rearrange("(kt p) t -> p kt t", kt=NKT, p=128)
                             [:, :, ch * 512:(ch + 1) * 512],
                    )
                    for pname, dst in (("wq", qT_sb), ("wk", kT_sb)):
                        pj = ppool.tile([128, 512], F32, name=f"p{pname}{ch}", tag=f"p{pname}")
                        for kt in range(NKT):
                            nc.tensor.matmul(
                                pj[:],
                                w_sb[pname][:, kt * 128:(kt + 1) * 128],
                                xs[:, kt * 512:(kt + 1) * 512],
                                start=(kt == 0), stop=(kt == NKT - 1),
                            )
                        nc.vector.tensor_copy(dst[:, ch * 512:(ch + 1) * 512], pj[:])
                    # v in [tok, d] layout: lhsT = x chunk subtile, rhs = wv
                    pv = ppool.tile([128, 512], F32, name=f"pv{ch}", tag="pv")
                    for sub in range(4):
                        for kt in range(NKT):
                            nc.tensor.matmul(
                                pv[:, sub * 128:(sub + 1) * 128],
                                xs[:, kt * 512 + sub * 128: kt * 512 + (sub + 1) * 128],
                                w_sb["wv"][:, kt * 128:(kt + 1) * 128],
                                start=(kt == 0), stop=(kt == NKT - 1),
                            )
                    nc.vector.tensor_copy(v_raw[:, ch * 512:(ch + 1) * 512], pv[:])

                # ---- rope on qT/kT (partition-shift via sbuf-sbuf DMA) ----
                cq = rope.tile([128, TOK], BF)
                nc.sync.dma_start(cq[:], cosqk[:])
                sq = rope.tile([128, TOK], BF)
                nc.sync.dma_start(sq[:], sinqk[:])
                for ti, t_sb in enumerate((qT_sb, kT_sb)):
                    # rot/tmp live on the same partitions as the rope rows
                    rot = rope.tile([128, TOK], BF, name=f"rot{ti}", tag="rot")
                    tmp = rope.tile([128, TOK], BF, name=f"rtmp{ti}", tag="rtmp")
                    for half in range(2):
                        t0, t1 = half * (TOK // 2), (half + 1) * (TOK // 2)
                        for h in range(2):
                            hb = h * 64
                            # rot[0:16]=t[16:32]; rot[16:32]=t[0:16] (sign in sinqk)
                            nc.sync.dma_start(rot[hb:hb + 16, t0:t1],
                                              t_sb[hb + 16:hb + 32, t0:t1])
                            nc.sync.dma_start(rot[hb + 16:hb + 32, t0:t1],
                                              t_sb[hb:hb + 16, t0:t1])
                        for h in range(2):
                            hb = h * 64
                            nc.vector.tensor_mul(
                                tmp[hb:hb + 32, t0:t1], rot[hb:hb + 32, t0:t1],
                                sq[hb:hb + 32, t0:t1])
                            nc.vector.tensor_mul(
                                t_sb[hb:hb + 32, t0:t1], t_sb[hb:hb + 32, t0:t1],
                                cq[hb:hb + 32, t0:t1])
                            nc.vector.tensor_add(
                                t_sb[hb:hb + 32, t0:t1], t_sb[hb:hb + 32, t0:t1],
                                tmp[hb:hb + 32, t0:t1])

                # ---- rope on v (free-axis strips; layout jt x h x d) ----
                cv = rope.tile([128, 64 * 64], BF)
                nc.sync.dma_start(cv[:], cosv[:])
                sv = rope.tile([128, 64 * 64], BF)
                nc.sync.dma_start(sv[:], sinv[:])
                vtmp = rope.tile([128, 64 * 128], BF)

                def vap(t, lo, w):
                    return t[:].rearrange("p (a h d) -> p a h d", a=64, h=2, d=64)[:, :, :, lo:lo + w]

                def vsap(t, lo, w):
                    return t[:].rearrange("p (a h d) -> p a h d", a=64, h=2, d=65)[:, :, :, lo:lo + w]

                def cap(t, lo, w):
                    return t[:].rearrange("p (a h d) -> p a h d", a=64, h=2, d=32)[:, :, :, lo:lo + w]

                # strip0 (d 0:16): v = raw0*cos0 + raw1*sin0   (sin0 = -sin)
                # strip1 (d 16:32): v = raw1*cos1 + raw0*sin1  (sin1 = +sin)
                nc.vector.tensor_mul(vsap(v_sb, 0, 16), vap(v_raw, 0, 16), cap(cv, 0, 16))
                nc.vector.tensor_mul(vsap(v_sb, 16, 16), vap(v_raw, 16, 16), cap(cv, 16, 16))
                nc.vector.tensor_mul(vap(vtmp, 0, 16), vap(v_raw, 16, 16), cap(sv, 0, 16))
                nc.vector.tensor_mul(vap(vtmp, 16, 16), vap(v_raw, 0, 16), cap(sv, 16, 16))
                nc.vector.tensor_add(vsap(v_sb, 0, 32), vsap(v_sb, 0, 32), vap(vtmp, 0, 32))
                nc.vector.tensor_copy(vsap(v_sb, 32, 32), vap(v_raw, 32, 32))
                # ones column for the fused row-sum in attn@v
                nc.vector.memset(
                    v_sb[:].rearrange("p (a h d) -> p a h d", a=64, h=2, d=65)
                        [:, :, :, 64:65], 1.0)

            # ---------------- phase 2: attention rounds ----------------
            # T-major: dotsT = k q^T per j-tile; exp (unnormalized) feeds
            # attn@v directly (ones column gives row sums in pav row 64);
            # normalization = reciprocal + partition-broadcast + TT multiply;
            # attn written j-major (host transposes during unshard).
            ag_in = [[dram.tile([64, N], BF, name=f"ag_in{g}_{h}") for h in range(2)]
                     for g in range(B)]
            ag_out = [[dram.tile([8 * 64, N], BF, name=f"ag_out{g}_{h}",
                                 addr_space="Shared") for h in range(2)]
                      for g in range(B)]

            with (
                tc.tile_pool(name="ring", bufs=24) as ring,
                tc.tile_pool(name="dps", bufs=2, space="PSUM") as dps,
                tc.tile_pool(name="avps", bufs=1, space="PSUM") as avps,
            ):
                for r in range(NR):
                    b, hl = r // 2, r % 2
                    hb = hl * 64
                    tok0 = b * N
                    pav = avps.tile([65, N], F32, name=f"pav{r}", tag="pav")
                    ats = []
                    for jt in range(NIT):
                        at = ring.tile([128, N], BF, name=f"at{r}_{jt}", tag="at")
                        ats.append(at)
                        for ih in range(2):
                            ps = dps.tile([128, 1024], F32, name=f"psd{r}_{jt}_{ih}",
                                          tag="psd")
                            for iq in range(2):
                                nc.tensor.matmul(
                                    ps[:, iq * 512:(iq + 1) * 512],
                                    kT_sb[hb:hb + 64, tok0 + jt * 128: tok0 + (jt + 1) * 128],
                                    qT_sb[hb:hb + 64, tok0 + ih * 1024 + iq * 512:
                                          tok0 + ih * 1024 + (iq + 1) * 512],
                                    start=True, stop=True,
                                )
                            nc.scalar.activation(
                                at[:, ih * 1024:(ih + 1) * 1024], ps[:],
                                AF.Exp, bias=0.0, scale=SCALE)
                        # attn@v (+row sums via ones col): accumulate over jt
                        for sub in range(4):
                            nc.tensor.matmul(
                                pav[:, sub * 512:(sub + 1) * 512],
                                v_sb[:, (b * NIT + jt) * 130 + hl * 65:
                                     (b * NIT + jt) * 130 + (hl + 1) * 65],
                                at[:, sub * 512:(sub + 1) * 512],
                                start=(jt == 0), stop=(jt == NIT - 1),
                            )
                    # round tail: softmax scales + normalize + writes
                    inv_row = ring.tile([1, N], BF, name=f"invr{r}", tag="invrow", bufs=4)
                    with nc.allow_low_precision(reason="softmax inv scale bf16"):
                        nc.vector.reciprocal(inv_row[:], pav[64:65, :])
                    inv_b = ring.tile([128, N], BF, name=f"invb{r}", tag="invb", bufs=4)
                    nc.gpsimd.partition_broadcast(inv_b[:], inv_row[:])
                    for jt in range(NIT):
                        nc.vector.tensor_mul(ats[jt][:], ats[jt][:], inv_b[:])
                        nc.sync.dma_start(
                            attn_outs[r][jt * 128:(jt + 1) * 128, :], ats[jt][:])
                    av = ring.tile([64, N], BF, name=f"av{r}", tag="av", bufs=4)
                    nc.vector.tensor_mul(av[:], pav[0:64, :], inv_b[0:64, :])
                    nc.gpsimd.dma_start(ag_in[b][hl][:], av[:])
                    nc.gpsimd.collective_compute(
                        "AllGather", mybir.AluOpType.bypass,
                        replica_groups=[list(range(N_CORES))],
                        ins=[ag_in[b][hl][:].opt()],
                        outs=[ag_out[b][hl][:].opt()],
                    )

            # ---------------- phase 3: output projection ----------------
            with (
                tc.tile_pool(name="ostage", bufs=3) as ostage,
                tc.tile_pool(name="ops", bufs=2, space="PSUM") as ops,
            ):
                for b in range(B):
                    for ch in range(4):
                        rs = ostage.tile([128, NKT * 512], BF, name=f"rs{b}_{ch}", tag="rs")
                        for hlf in range(2):
                            nc.sync.dma_start(
                                rs[:, hlf * 4 * 512:(hlf + 1) * 4 * 512],
                                ag_out[b][hlf][:].rearrange(
                                    "(t p) i -> p t i", t=4, p=128)
                                    [:, :, ch * 512:(ch + 1) * 512],
                            )
                        po = ops.tile([128, 512], F32, name=f"po{b}_{ch}", tag="po")
                        for kt in range(NKT):
                            nc.tensor.matmul(
                                po[:],
                                w_sb["wo"][:, kt * 128:(kt + 1) * 128],
                                rs[:, kt * 512:(kt + 1) * 512],
                                start=(kt == 0), stop=(kt == NKT - 1),
                            )
                        xr = ostage.tile([128, 512], F32, name=f"xr{b}_{ch}", tag="xr")
                        nc.sync.dma_start(
                            xr[:], xresT[:, b * N + ch * 512: b * N + (ch + 1) * 512])
                        osb = ostage.tile([128, 512], F32, name=f"osb{b}_{ch}", tag="osb")
                        nc.vector.tensor_scalar_add(osb[:], po[:], bo_sb[:, 0:1])
                        nc.vector.tensor_add(osb[:], osb[:], xr[:])
                        nc.sync.dma_start(
                            outT[:, b * N + ch * 512: b * N + (ch + 1) * 512], osb[:])

    nc.compile()
    return nc


def _host_inputs(x, rotary_pos_emb, Wq, Wk, Wv, Wo, bo):
    """Build the 8 per-core input maps (host-side sharding/layout prep)."""
    bf = ml_dtypes.bfloat16
    xf = np.ascontiguousarray(x.reshape(TOK, DIM))
    xT_f32 = np.ascontiguousarray(xf.T)            # [DIM, TOK]
    xT_bf = xT_f32.astype(bf)

    # rope tables from rotary_pos_emb [1,1,N,32] = concat(f_half, f_half)
    f_half = np.asarray(rotary_pos_emb[0, 0, :, :16], dtype=np.float64)  # [N, 16]
    ch = np.cos(f_half).astype(np.float32)
    sh = np.sin(f_half).astype(np.float32)

    # qk (T layout): rows 0:16 -> cos/-sin, 16:32 -> cos/+sin, dup at rows 64..
    cos32 = np.concatenate([ch.T, ch.T], axis=0)           # [32, N]
    sin32 = np.concatenate([-sh.T, sh.T], axis=0)          # [32, N]
    cosqk = np.zeros((128, TOK), np.float32)
    sinqk = np.zeros((128, TOK), np.float32)
    for hh in range(2):
        cosqk[hh * 64:hh * 64 + 32] = np.tile(cos32, (1, B))
        sinqk[hh * 64:hh * 64 + 32] = np.tile(sin32, (1, B))
    cosqk = cosqk.astype(bf)
    sinqk = sinqk.astype(bf)

    # v (token-major layout): per flat-token-tile blocks [h(2) x d(32)]
    pos = np.arange(TOK) % N
    cblk = np.concatenate([ch, ch], axis=1)[pos]           # [TOK, 32]
    sblk = np.concatenate([-sh, sh], axis=1)[pos]          # [TOK, 32]
    cosv = np.concatenate([cblk, cblk], axis=1)            # [TOK, 64] (h dup)
    sinv = np.concatenate([sblk, sblk], axis=1)
    cosv = cosv.reshape(64, 128, 64).transpose(1, 0, 2).reshape(128, 64 * 64).astype(bf)
    sinv = sinv.reshape(64, 128, 64).transpose(1, 0, 2).reshape(128, 64 * 64).astype(bf)

    def wslice(W, c):
        wc = W[:, c * 128:(c + 1) * 128]                   # [1024, 128]
        return np.ascontiguousarray(
            wc.reshape(NKT, 128, 128).transpose(1, 0, 2).reshape(128, NKT * 128)
        ).astype(bf)

    def woslice(W, c):
        # contraction tile (hlf, t) row p maps to global hd =
        # 256*t + (p//64)*128 + hlf*64 + (p%64)
        wc = W[:, c * 128:(c + 1) * 128]                   # [1024, 128]
        rows = np.empty((2, 4, 128), np.int64)
        p = np.arange(128)
        for hlf in range(2):
            for t in range(4):
                rows[hlf, t] = 256 * t + (p // 64) * 128 + hlf * 64 + (p % 64)
        perm = wc[rows.reshape(-1)]                        # [8*128, 128]
        return np.ascontiguousarray(
            perm.reshape(NKT, 128, 128).transpose(1, 0, 2).reshape(128, NKT * 128)
        ).astype(bf)

    in_maps = []
    for c in range(N_CORES):
        in_maps.append({
            "xT": xT_bf,
            "wq": wslice(np.asarray(Wq), c),
            "wk": wslice(np.asarray(Wk), c),
            "wv": wslice(np.asarray(Wv), c),
            "wo": woslice(np.asarray(Wo), c),
            "bo": np.ascontiguousarray(np.asarray(bo)[c * 128:(c + 1) * 128]
                                       ).reshape(128, 1).astype(np.float32),
            "xresT": np.ascontiguousarray(xT_f32[c * 128:(c + 1) * 128]),
            "cosqk": cosqk, "sinqk": sinqk, "cosv": cosv, "sinv": sinv,
        })
    return in_maps


def run(inputs, trace=False, trace_kwargs=None):
    """Run the SPMD kernel; returns (results_list, exec_time_ns)."""
    global _BUILT
    if _BUILT is None:
        _BUILT = build()
    in_maps = _host_inputs(**inputs)
    kw = {}
    if trace:
        kw["trace"] = True
        if trace_kwargs:
            kw.update(trace_kwargs)
    res = run_bass_kernel_spmd(_BUILT, in_maps, core_ids=list(range(N_CORES)), **kw)
    return res.results, res.exec_time_ns


def assemble_outputs(results):
    out = np.empty((B, N, DIM), np.float32)
    attn = np.empty((B, HEADS, N, N), np.float32)
    for c in range(N_CORES):
        rr = results[c]
        out.reshape(TOK, DIM)[:, c * 128:(c + 1) * 128] = rr["outT"].T
        for r in range(NR):
            b, hl = r // 2, r % 2
            attn[b, 2 * c + hl] = np.asarray(rr[f"attn{r}"]).T.astype(np.float32)
    return out, attn


def kernel(**inputs):
    results, _ = run(inputs)
    return assemble_outputs(results)
